# revision 33
# baseline (speedup 1.0000x reference)
"""FourierLinear Trainium2 kernel v6 — host tables + mod-4 folds both stages.

Stage 1 (as v5): u_f[m] = sum_k trig(w a_f k) x[k,m] folded by a_f mod 4:
  a%4==0 / a%4==2 contract k'<1024 against (x0+x1+x2+x3) / (x0-x1+x2-x3);
  odd a contracts k'<2048 against (x_lo - x_hi).  Chunks may carry two
  table sets (mixed chunks merge the even-a group remainders, conserving
  stage-1 time while saving whole stage-2 chunks).

Stage 2 (new): mod-4 l-fold.  For l' < 1024 accumulate four psum partials
per row block:
  P0 (b%4==0), P2 (b%4==2):  sum s(uc cos - us sin)(w b l')
  B  (b odd):                sum s(uc cos - us sin)
  D  (b odd):                sum sigma s(uc sin + us cos), sigma=+1 b%4==3
then with A = P0+P2, C = P0-P2:
  y[l']      = A+B   y[l'+1024] = C+D   y[l'+2048] = A-B   y[l'+3072] = C-D
Chunks must be b-class pure (b%4 in {0,2} or b odd) — the 3x3 (a-fold,
b-class) grouping plus remainder merging gives 19 chunks here vs 19 for
the v5 mod-2 scheme, but stage-2 matmul time drops 26% (even-b chunks
touch half the l' range) and the yE park/drain phase disappears (4 banks
per row block, double-buffered).  Stage-2 tables are SBUF-resident per
l'-position, loaded once and reused across all 8 row blocks.

The 2^-16 ifft2 norm (* 256) stays split: 2^-8 at the stage-1 psum copy,
2^-8 inside the stage-2 tables.
"""

import numpy as np

import concourse.mybir as mybir
import concourse.tile as tile
from concourse import bacc
from concourse.bass_utils import run_bass_kernel_spmd

N_CORES = 8
IN_F = 4096
OUT_F = 4096
NF = 2048
ROWS = 8192
M = ROWS // N_CORES   # 1024 rows per core
P = 128
KH = IN_F // 2        # 2048 folded k' (odd-a)
KQ = IN_F // 4        # 1024 quarter-folded k' (even-a)
LQ = OUT_F // 4       # 1024 quarter-folded l' range
NT = 512
NPOS = LQ // NT       # 2 l'-positions
MS = M // P           # 8 row blocks
KCH_MAX = KH // P     # 16

LAST_RESULTS = None
_NC_CACHE = None


def _build_nc(desc):
    # desc: tuple of (bcl, sets) per 128-lane f-chunk, sorted bcl-major.
    #   bcl: 0 -> b%4==0 (P0), 1 -> b%4==2 (P2), 2 -> b odd (B and D)
    #   sets: tuple of (kch, xsel); xsel 0 -> xA0, 1 -> xA2, 2 -> xm
    NCH = len(desc)
    odd_ids = [i for i, (bcl, _) in enumerate(desc) if bcl == 2]
    NODD = len(odd_ids)
    oidx = {ci: k for k, ci in enumerate(odd_ids)}
    koff = [0]
    for _, sets in desc:
        koff.append(koff[-1] + sum(kch for kch, _ in sets))
    TBLK = koff[-1]
    # per-psum-class chunk ranges (for matmul start/stop flags)
    cls_ids = {c: [i for i, (bcl, _) in enumerate(desc) if bcl == c]
               for c in range(3)}
    f32 = mybir.dt.float32
    f16 = mybir.dt.float16
    mult = mybir.AluOpType.mult
    add = mybir.AluOpType.add
    sub = mybir.AluOpType.subtract

    nc = bacc.Bacc(None)
    xA0T = nc.declare_dram_parameter("xA0T", [KQ, M], f16, isOutput=False)
    xA2T = nc.declare_dram_parameter("xA2T", [KQ, M], f16, isOutput=False)
    xmT = nc.declare_dram_parameter("xmT", [KH, M], f16, isOutput=False)
    t1c = nc.declare_dram_parameter("t1c", [TBLK * P, P], f16, isOutput=False)
    t1s = nc.declare_dram_parameter("t1s", [TBLK * P, P], f16, isOutput=False)
    t2pc = nc.declare_dram_parameter("t2pc", [NCH * NPOS * P, NT], f16,
                                     isOutput=False)
    t2ps = nc.declare_dram_parameter("t2ps", [NCH * NPOS * P, NT], f16,
                                     isOutput=False)
    t2dc = nc.declare_dram_parameter("t2dc", [NODD * NPOS * P, NT], f16,
                                     isOutput=False)
    t2ds = nc.declare_dram_parameter("t2ds", [NODD * NPOS * P, NT], f16,
                                     isOutput=False)
    out = nc.declare_dram_parameter("out", [M, OUT_F], f16, isOutput=True)

    xA0p = xA0T[:].rearrange("(kc p) m -> p kc m", p=P)
    xA2p = xA2T[:].rearrange("(kc p) m -> p kc m", p=P)
    xmp = xmT[:].rearrange("(kc p) m -> p kc m", p=P)
    t1cp = t1c[:].rearrange("(blk p) j -> p blk j", p=P)
    t1sp = t1s[:].rearrange("(blk p) j -> p blk j", p=P)
    t2pcp = t2pc[:].rearrange("(ch ps p) l -> p ch ps l", ch=NCH, ps=NPOS, p=P)
    t2psp = t2ps[:].rearrange("(ch ps p) l -> p ch ps l", ch=NCH, ps=NPOS, p=P)
    t2dcp = t2dc[:].rearrange("(ch ps p) l -> p ch ps l", ch=NODD, ps=NPOS, p=P)
    t2dsp = t2ds[:].rearrange("(ch ps p) l -> p ch ps l", ch=NODD, ps=NPOS, p=P)
    outp = out[:].rearrange("(ms p) n -> p ms n", p=P)

    with tile.TileContext(nc) as tc:
        with (
            tc.tile_pool(name="v", bufs=1) as vpool,
            tc.tile_pool(name="o", bufs=2) as opool,
        ):
            vc = vpool.tile([P, NCH, M], f16)   # u_cos * 2^-8
            vs = vpool.tile([P, NCH, M], f16)
            # pos-0 stage-2 tables live outside the stage-1 pools so their
            # DMAs (emitted at the end of stage 1, behind its queue traffic)
            # can stream during stage-1 compute instead of stalling the
            # transition on the freed-region WAR.
            tpa0 = vpool.tile([P, NCH, NT], f16, tag="tpa0")
            tpb0 = vpool.tile([P, NCH, NT], f16, tag="tpb0")

            # ---- stage 1
            with (
                tc.tile_pool(name="x", bufs=1) as xpool,
                tc.tile_pool(name="t1", bufs=2) as t1p,
                tc.tile_pool(name="ps1", bufs=4, space="PSUM") as ps1,
            ):
                xA0 = xpool.tile([P, KQ // P, M], f16)
                xA2 = xpool.tile([P, KQ // P, M], f16)
                xm = xpool.tile([P, KCH_MAX, M], f16)
                xtiles = [xA0, xA2, xm]
                xaps = [xA0p, xA2p, xmp]

                def tbl_tiles(i):
                    tot = koff[i + 1] - koff[i]
                    tcb = t1p.tile([P, KCH_MAX, P], f16, tag="tc")
                    tsb = t1p.tile([P, KCH_MAX, P], f16, tag="ts")
                    nc.sync.dma_start(
                        tcb[:, 0:tot, :], t1cp[:, koff[i] : koff[i] + tot, :])
                    nc.scalar.dma_start(
                        tsb[:, 0:tot, :], t1sp[:, koff[i] : koff[i] + tot, :])
                    return tcb, tsb

                # x-piece DMAs in first-use order, emitted interleaved with
                # the per-chunk table DMAs: queue FIFO order then delivers
                # chunk i+1's tables between x pieces instead of parking all
                # table traffic behind the full 8 MB x stream.
                NPRE = 2
                pre = [tbl_tiles(i) for i in range(NPRE)]
                first_xsel = desc[0][1][0][1]
                xorder = [first_xsel] + [i for i in range(3) if i != first_xsel]
                xkch = [KQ // P, KQ // P, KCH_MAX]
                xq = [(xi, kc) for xi in xorder for kc in range(xkch[xi])]
                qi = 0

                def emit_x(n):
                    nonlocal qi
                    for _ in range(min(n, len(xq))):
                        xi, kc = xq.pop(0)
                        eng = nc.sync if qi % 2 == 0 else nc.scalar
                        eng.dma_start(xtiles[xi][:, kc, :], xaps[xi][:, kc, :])
                        qi += 1

                emit_x(12)
                for i, (bcl, sets) in enumerate(desc):
                    tot = koff[i + 1] - koff[i]
                    tcb, tsb = pre[i] if i < NPRE else tbl_tiles(i)
                    emit_x(3)
                    psc = ps1.tile([P, M], f32, tag="u", name=f"psc{i}")
                    pss = ps1.tile([P, M], f32, tag="u", name=f"pss{i}")
                    bi = 0
                    for kch, xsel in sets:
                        xf = xtiles[xsel]
                        for kc in range(kch):
                            st, sp = bi == 0, bi == tot - 1
                            nc.tensor.matmul(psc[:, 0:NT], tcb[:, bi, :],
                                             xf[:, kc, 0:NT], start=st, stop=sp)
                            nc.tensor.matmul(psc[:, NT:M], tcb[:, bi, :],
                                             xf[:, kc, NT:M], start=st, stop=sp)
                            nc.tensor.matmul(pss[:, 0:NT], tsb[:, bi, :],
                                             xf[:, kc, 0:NT], start=st, stop=sp)
                            nc.tensor.matmul(pss[:, NT:M], tsb[:, bi, :],
                                             xf[:, kc, NT:M], start=st, stop=sp)
                            bi += 1
                    nc.scalar.mul(vc[:, i, :], psc[:], 2.0 ** -8)
                    nc.vector.tensor_scalar(vs[:, i, :], pss[:], 2.0 ** -8,
                                            None, mult)
                for ci in range(NCH):
                    nc.sync.dma_start(tpa0[:, ci, :], t2pcp[:, ci, 0, :])
                    nc.scalar.dma_start(tpb0[:, ci, :], t2psp[:, ci, 0, :])

            # ---- stage 2: four partials per (pos, ms), combine, write 4 blocks
            with (
                tc.tile_pool(name="t2p", bufs=1) as t2pp,
                tc.tile_pool(name="t2d", bufs=1) as t2dp,
                tc.tile_pool(name="ac", bufs=2) as acp,
                tc.tile_pool(name="ps2", bufs=2, space="PSUM") as ps2,
            ):
                for pos in range(NPOS):
                    # per-chunk sliced loads in consumption order: matmuls
                    # start as soon as the first chunks land instead of
                    # waiting for one monolithic multi-MB transfer (which
                    # also sits behind the WAR on the freed stage-1 region)
                    if pos == 0:
                        tpa, tpb = tpa0, tpb0
                    else:
                        tpa = t2pp.tile([P, NCH, NT], f16, tag="tpa")
                        tpb = t2pp.tile([P, NCH, NT], f16, tag="tpb")
                        for ci in range(NCH):
                            nc.sync.dma_start(tpa[:, ci, :],
                                              t2pcp[:, ci, pos, :])
                            nc.scalar.dma_start(tpb[:, ci, :],
                                                t2psp[:, ci, pos, :])
                    tda = t2dp.tile([P, NODD, NT], f16, tag="tda")
                    tdb = t2dp.tile([P, NODD, NT], f16, tag="tdb")
                    for oi in range(NODD):
                        nc.gpsimd.dma_start(tda[:, oi, :], t2dcp[:, oi, pos, :])
                        nc.gpsimd.dma_start(tdb[:, oi, :], t2dsp[:, oi, pos, :])
                    for ms in range(MS):
                        pp = [ps2.tile([P, NT], f32, tag=f"pp{c}",
                                       name=f"pp{pos}_{ms}_{c}")
                              for c in range(3)]
                        dd = ps2.tile([P, NT], f32, tag="dd",
                                      name=f"dd{pos}_{ms}")
                        msl = slice(ms * P, (ms + 1) * P)
                        # walk chunks in ci order (the order the resident
                        # table slices stream in); each chunk's matmuls land
                        # in its own class psum, classes interleave freely
                        for ci in range(NCH):
                            c = desc[ci][0]
                            ids = cls_ids[c]
                            nc.tensor.matmul(
                                pp[c][:], vc[:, ci, msl], tpa[:, ci, :],
                                start=(ci == ids[0]), stop=False)
                            if c == 2:
                                oi = oidx[ci]
                                nc.tensor.matmul(
                                    dd[:], vc[:, ci, msl], tda[:, oi, :],
                                    start=(oi == 0), stop=False)
                            nc.tensor.matmul(
                                pp[c][:], vs[:, ci, msl], tpb[:, ci, :],
                                start=False, stop=(ci == ids[-1]))
                            if c == 2:
                                nc.tensor.matmul(
                                    dd[:], vs[:, ci, msl], tdb[:, oi, :],
                                    start=False, stop=(oi == NODD - 1))
                        # A = P0+P2, C = P0-P2; y_j = A+-B / C+-D.  DVE may
                        # read only one PSUM operand, so P0 goes to SBUF
                        # first (on the otherwise-idle scalar engine).
                        p0sb = acp.tile([P, NT], f32, tag="p0", name="p0sb")
                        at = acp.tile([P, NT], f32, tag="A", name="at")
                        ct = acp.tile([P, NT], f32, tag="C", name="ct")
                        nc.scalar.copy(out=p0sb[:], in_=pp[0][:])
                        nc.vector.tensor_tensor(out=at[:], in0=p0sb[:],
                                                in1=pp[1][:], op=add)
                        nc.vector.tensor_tensor(out=ct[:], in0=p0sb[:],
                                                in1=pp[1][:], op=sub)
                        for j, (lhs, ps, op) in enumerate(
                            ((at, pp[2], add), (ct, dd, add),
                             (at, pp[2], sub), (ct, dd, sub))):
                            ot = opool.tile([P, NT], f16, tag=f"y{j}",
                                            name=f"y{j}")
                            nc.vector.tensor_tensor(out=ot[:], in0=lhs[:],
                                                    in1=ps[:], op=op)
                            col = pos * NT + LQ * j
                            eng = nc.scalar if j % 2 == 0 else nc.sync
                            eng.dma_start(outp[:, ms, col : col + NT], ot[:])
    nc.finalize()
    return nc


def _host_prep(x, spectrum, indices):
    x2 = np.asarray(x, dtype=np.float32).reshape(ROWS, IN_F)
    idx = np.asarray(indices, dtype=np.int64)
    s = np.asarray(spectrum, dtype=np.float32)
    a, b = idx[0], idx[1]

    # reference scatter is last-write-wins on duplicate (a,b) pairs
    keys = a * OUT_F + b
    _, first_of_reversed = np.unique(keys[::-1], return_index=True)
    keep = np.zeros(NF, dtype=bool)
    keep[NF - 1 - first_of_reversed] = True
    s_eff = np.where(keep, s, 0.0).astype(np.float32)

    # chunks: 3x3 (a-fold, b-class) groups, even-a remainders merged per
    # b-class into mixed chunks; odd-a groups padded to >= 1 chunk.
    afold = [(lambda v: v % 4 == 0, 8, 0, 0),
             (lambda v: v % 4 == 2, 8, 1, 2),
             (lambda v: v % 2 == 1, 16, 2, 1)]
    bklass = [(lambda v: v % 4 == 0, 0), (lambda v: v % 4 == 2, 2),
              (lambda v: v % 2 == 1, 1)]
    # even-a chunks first (they only need the quarter-folded x streams),
    # odd-a chunks last so the larger xm stream has time to arrive;
    # stage-2 addresses chunks via explicit class id lists, so order is free
    chunk_f, chunk_xsel, desc = [], [], []
    odd_f, odd_xsel, odd_desc = [], [], []
    for bcl, (bsel, db) in enumerate(bklass):
        pool_f, pool_xs, pool_da = [], [], []
        for asel, kch, xsel, da in afold[:2]:
            sel = np.nonzero(asel(a) & bsel(b))[0]
            nfull = len(sel) // P
            for c in range(nfull):
                chunk_f.append(sel[c * P : (c + 1) * P])
                chunk_xsel.append([xsel] * P)
                desc.append((bcl, ((8, xsel),)))
            pool_f += list(sel[nfull * P :])
            pool_xs += [xsel] * (len(sel) - nfull * P)
            pool_da += [da] * (len(sel) - nfull * P)
        for c0 in range(0, len(pool_f), P):
            fi = pool_f[c0 : c0 + P]
            xsi = pool_xs[c0 : c0 + P]
            pad = P - len(fi)
            fi = np.array(fi + [-1] * pad)
            xsi = xsi + [xsi[0]] * pad
            chunk_f.append(fi)
            chunk_xsel.append(xsi)
            desc.append((bcl, tuple((8, xs) for xs in sorted(set(xsi)))))
        asel, kch, xsel, da = afold[2]
        sel = np.nonzero(asel(a) & bsel(b))[0]
        n = max(1, -(-len(sel) // P))
        selp = np.concatenate([sel, -np.ones(n * P - len(sel), np.int64)])
        for c in range(n):
            odd_f.append(selp[c * P : (c + 1) * P])
            odd_xsel.append([2] * P)
            odd_desc.append((bcl, ((16, 2),)))
    chunk_f += odd_f
    chunk_xsel += odd_xsel
    desc += odd_desc
    # build padded lane arrays;
    # dummy lanes (f == -1) get parity-consistent a/b and zero spectrum
    dummy_ab = {0: (0, 0), 1: (0, 2), 2: (1, 1)}
    NCH = len(desc)
    a2 = np.zeros(NCH * P, np.int64)
    b2 = np.zeros(NCH * P, np.int64)
    s2 = np.zeros(NCH * P, np.float32)
    for i in range(NCH):
        bcl = desc[i][0]
        fi = chunk_f[i]
        for j in range(P):
            if fi[j] >= 0:
                a2[i * P + j] = a[fi[j]]
                b2[i * P + j] = b[fi[j]]
                s2[i * P + j] = s_eff[fi[j]]
            else:
                da = 0 if chunk_xsel[i][j] == 0 else (2 if chunk_xsel[i][j] == 1 else 1)
                a2[i * P + j] = da
                b2[i * P + j] = dummy_ab[bcl][1]

    w = 2.0 * np.pi / 4096.0
    # stage-1 tables: per chunk, one block-set per (kch, xsel) table set;
    # lanes not in the set get zero rows
    t1c_parts, t1s_parts = [], []
    for i, (bcl, sets) in enumerate(desc):
        al = a2[i * P : (i + 1) * P]
        xsl = np.array(chunk_xsel[i])
        for kch, xsel in sets:
            kq = np.arange(kch * P)
            ph = (al[None, :] * kq[:, None]) % 4096
            mask = (xsl == xsel)[None, :]
            t1c_parts.append(np.where(mask, np.cos(w * ph), 0.0).astype(np.float32))
            t1s_parts.append(np.where(mask, np.sin(w * ph), 0.0).astype(np.float32))
    t1c = np.ascontiguousarray(
        np.concatenate(t1c_parts, axis=0).astype(np.float16))
    t1s = np.ascontiguousarray(
        np.concatenate(t1s_parts, axis=0).astype(np.float16))

    # stage-2 tables over l' < 1024, s*2^-8 folded in
    ll = np.arange(LQ)
    odd_ids = [i for i, (bcl, _) in enumerate(desc) if bcl == 2]
    NODD = len(odd_ids)
    t2pc = np.zeros((NCH, NPOS, P, NT), np.float32)
    t2ps = np.zeros((NCH, NPOS, P, NT), np.float32)
    t2dc = np.zeros((NODD, NPOS, P, NT), np.float32)
    t2ds = np.zeros((NODD, NPOS, P, NT), np.float32)
    oi = 0
    for i, (bcl, _) in enumerate(desc):
        bl = b2[i * P : (i + 1) * P]
        sc = (s2[i * P : (i + 1) * P] * 2.0 ** -8)[:, None]
        ph = (bl[:, None] * ll[None, :]) % 4096
        cosb = np.cos(w * ph, dtype=np.float32)
        sinb = np.sin(w * ph, dtype=np.float32)
        t2pc[i] = (cosb * sc).reshape(P, NPOS, NT).transpose(1, 0, 2)
        t2ps[i] = (-sinb * sc).reshape(P, NPOS, NT).transpose(1, 0, 2)
        if bcl == 2:
            sg = np.where(bl % 4 == 3, 1.0, -1.0)[:, None]
            t2dc[oi] = (sinb * sc * sg).reshape(P, NPOS, NT).transpose(1, 0, 2)
            t2ds[oi] = (cosb * sc * sg).reshape(P, NPOS, NT).transpose(1, 0, 2)
            oi += 1
    t2pc = np.ascontiguousarray(t2pc.reshape(NCH * NPOS * P, NT).astype(np.float16))
    t2ps = np.ascontiguousarray(t2ps.reshape(NCH * NPOS * P, NT).astype(np.float16))
    t2dc = np.ascontiguousarray(t2dc.reshape(NODD * NPOS * P, NT).astype(np.float16))
    t2ds = np.ascontiguousarray(t2ds.reshape(NODD * NPOS * P, NT).astype(np.float16))

    q0, q1, q2, q3 = (x2[:, i * KQ : (i + 1) * KQ] for i in range(4))
    xA0 = (q0 + q1 + q2 + q3).astype(np.float16)
    xA2 = (q0 - q1 + q2 - q3).astype(np.float16)
    xm16 = (x2[:, :KH] - x2[:, KH:]).astype(np.float16)
    tabs = {"t1c": t1c, "t1s": t1s, "t2pc": t2pc, "t2ps": t2ps,
            "t2dc": t2dc, "t2ds": t2ds}
    return xA0, xA2, xm16, tabs, tuple(desc)


def kernel(x, spectrum, indices):
    global _NC_CACHE, LAST_RESULTS
    xA0, xA2, xm16, tabs, desc = _host_prep(x, spectrum, indices)

    if _NC_CACHE is None or _NC_CACHE[0] != desc:
        _NC_CACHE = (desc, _build_nc(desc))
    nc = _NC_CACHE[1]

    in_maps = [
        {
            "xA0T": np.ascontiguousarray(xA0[j * M : (j + 1) * M].T),
            "xA2T": np.ascontiguousarray(xA2[j * M : (j + 1) * M].T),
            "xmT": np.ascontiguousarray(xm16[j * M : (j + 1) * M].T),
            **tabs,
        }
        for j in range(N_CORES)
    ]
    res = run_bass_kernel_spmd(nc, in_maps, list(range(N_CORES)))
    LAST_RESULTS = res
    out = np.concatenate(
        [res.results[j]["out"].astype(np.float32) for j in range(N_CORES)], axis=0
    )
    return out.reshape(np.asarray(x).shape[:-1] + (OUT_F,))
